# revision 10
# baseline (speedup 1.0000x reference)
"""Trainium2 Bass kernel for nn_LASCC (sparse patch-correlation attention + top-k).

Math (per batch element b):
  x_hat = L2-normalize(x, dim=channels)
  z_p[c, n] = x_hat at the two in-patch diagonal pixels (p=0: (0,0), p=1: (1,1))
  C_p = z_p^T z_p                  (1024x1024 normalized correlation, symmetric)
  C_2 = (C_0 + C_1)/2              (avg map)
  A_q = softmax_row * softmax_col = E^2/(r[n] r[m]),  E=exp(alpha*mask*C), r=rowsum(E)
  out pixel with patch n, map q: top-3 over m of A_q[n, m]
    = (top-3 over m of E[n,m]*sqrt(u[m]))^2 * u[n],  u = 1/r

v5 design (v4 post-mortem: merging the three maps into one E-superphase made
the scalar engine the in-stage serializer -- 57us/batch with F unable to
overlap; norm phase lost 30us to sync-queue HOL blocking and 485ns/op gpsimd
dependency chains):
  - six per-(b,q) stages, software-pipelined E(k+1) || F(k) so the scalar
    E-phase (~12us) and the DVE F-phase (~16us) of consecutive stages overlap.
  - E stored once as fp16 slab (exp + rowsum accum); F is v = E * bcast
    (1/sqrt(r)) (DVE fp16 2x) + MAX8 + two batched stride-0 finals.
  - all rsqrts via bit-hack + Newton (no Ln/Sqrt: ONE ACT table set for the
    whole kernel). Latency-critical chains run on DVE (~200ns/op); the
    off-path b1 norm chain and the avg-map adds run on gpsimd.
  - channel norms in [128, 16] transposed layout (16 small matmuls/batch),
    inverse norms broadcast via DMA partition-broadcast, both batches in two
    chains; x/mask DMAs issued before any dependent DMA (sync is in-order).
  - avg map: s2 = s0+s1 on gpsimd per chunk (its E-phase is gpsimd+scalar
    only and overlaps F(b,q1) on the DVE).
"""
import numpy as np

import concourse.bass as bass
import concourse.mybir as mybir
from concourse import bacc
from concourse.tile import TileContext
from concourse.bass_utils import run_bass_kernel_spmd

F32 = mybir.dt.float32
F32R = mybir.dt.float32r
F16 = mybir.dt.float16
I32 = mybir.dt.int32
AF = mybir.ActivationFunctionType
ALU = mybir.AluOpType

B_FULL = 16
N_CORES = 8
B_LOC = B_FULL // N_CORES  # 2
C = 128
H = W = 64
NPH = 32
NP = 1024  # patches
PS = 2
TOPK = 3
NCHUNK = NP // 128  # 8

LAST_EXEC_NS = None


def _build_mask() -> np.ndarray:
    """(1 - gaussian) self-suppression mask, [NP, NP] (matches reference)."""
    rat_s = np.float32(0.05)
    sr = np.float32(NPH) * rat_s
    ind_r = np.arange(NPH, dtype=np.float32).reshape(1, NPH, 1)
    ind_c = np.arange(NPH, dtype=np.float32).reshape(1, 1, NPH)
    cent = np.arange(NPH, dtype=np.float32)
    cent_r = np.repeat(cent, NPH).reshape(NP, 1, 1)
    cent_c = np.tile(cent, NPH).reshape(NP, 1, 1)
    g = np.exp(-((ind_r - cent_r) ** 2) / (2.0 * sr * sr)) * np.exp(
        -((ind_c - cent_c) ** 2) / (2.0 * sr * sr)
    )
    return (1.0 - g).reshape(NP, NP)


def build_nc():
    nc = bacc.Bacc(trn_type="TRN2")

    x_d = nc.dram_tensor("x", [B_LOC, C, H * W], F32, kind="ExternalInput")
    mask_d = nc.dram_tensor("mask", [NP, NP], F16, kind="ExternalInput")
    alpha_d = nc.dram_tensor("alpha", [128, 1], F32, kind="ExternalInput")
    out_d = nc.dram_tensor("out", [B_LOC, 3, NP, TOPK], F32, kind="ExternalOutput")

    with TileContext(nc) as tc:
        with tc.tile_pool(name="const", bufs=1) as cpool, \
             tc.tile_pool(name="slab", bufs=4) as slabp, \
             tc.tile_pool(name="eslab", bufs=3) as epool, \
             tc.tile_pool(name="z", bufs=1) as zpool, \
             tc.tile_pool(name="work", bufs=3) as work, \
             tc.tile_pool(name="small", bufs=3) as small, \
             tc.tile_pool(name="ps", bufs=3, space="PSUM") as ps, \
             tc.tile_pool(name="psn", bufs=2, space="PSUM") as psn, \
             tc.tile_pool(name="dsc", bufs=4, space="DRAM") as dsc:

            # ---- input DMAs first: sync is in-order, so nothing dependent
            # may be queued ahead of the big loads.
            xs = {}
            for b in range(B_LOC):
                xs[b] = slabp.tile([128, H * W], F32, name=f"xs{b}",
                                   tag="slab16")
                nc.sync.dma_start(xs[b], x_d[b])
            mask_sb = cpool.tile([128, NCHUNK, NP], F16)
            nc.sync.dma_start(
                mask_sb, mask_d[:, :].rearrange("(i p) m -> p i m", p=128)
            )
            av = cpool.tile([128, 1], F32)  # alpha
            nc.sync.dma_start(av, alpha_d[:, :])

            # ---- constants
            ones_k = cpool.tile([128, 1], F32)
            nc.vector.memset(ones_k, 1.0)
            av_h = cpool.tile([128, 1], F32)
            nc.vector.tensor_scalar_mul(av_h, av, 0.5)
            scale_E = [av, av, av_h]
            c15 = cpool.tile([128, 16], F32)
            nc.vector.memset(c15, 1.5)
            chalf = cpool.tile([128, 16], F32)
            nc.vector.memset(chalf, 0.5)
            cone_i = cpool.tile([128, 16], I32)
            nc.vector.memset(cone_i, 1)
            cmagic = cpool.tile([128, 16], I32)
            nc.vector.memset(cmagic, 0x5F3759DF)

            def emit_rsqrt(src, out_dtype, tag, iters, newton_eng):
                """1/sqrt(src) on [128,k<=16]: DVE bit-hack seed + Newton
                iterations on `newton_eng` (vector for latency-critical
                chains, gpsimd for off-path ones)."""
                k = src.shape[-1]
                yi = small.tile([128, k], I32, name="yi", tag=f"yi{tag}")
                nc.vector.tensor_tensor(out=yi, in0=src.bitcast(I32),
                                        in1=cone_i[:, :k],
                                        op=ALU.logical_shift_right)
                y0i = small.tile([128, k], I32, name="y0i", tag=f"y0{tag}")
                nc.vector.tensor_tensor(out=y0i, in0=cmagic[:, :k], in1=yi,
                                        op=ALU.subtract)
                cur = y0i.bitcast(F32)
                for it in range(iters):
                    t1 = small.tile([128, k], F32, name="t1", tag=f"t1{tag}")
                    newton_eng.tensor_tensor(out=t1, in0=cur, in1=cur,
                                             op=ALU.mult)
                    t2 = small.tile([128, k], F32, name="t2", tag=f"t2{tag}")
                    newton_eng.tensor_tensor(out=t2, in0=t1, in1=src,
                                             op=ALU.mult)
                    t2h = small.tile([128, k], F32, name="t2h",
                                     tag=f"t2h{tag}")
                    newton_eng.tensor_tensor(out=t2h, in0=t2, in1=chalf[:, :k],
                                             op=ALU.mult)
                    t3 = small.tile([128, k], F32, name="t3", tag=f"t3{tag}")
                    newton_eng.tensor_tensor(out=t3, in0=c15[:, :k], in1=t2h,
                                             op=ALU.subtract)
                    odt = out_dtype if it == iters - 1 else F32
                    t4 = small.tile([128, k], odt, name="t4",
                                    tag=f"t4{tag}{it}")
                    newton_eng.tensor_tensor(out=t4, in0=cur, in1=t3,
                                             op=ALU.mult)
                    cur = t4
                return cur

            # ---- phase N: channel norms + normalized z, one batched chain
            # per batch element. b0's chain runs on DVE (gates stage 0), b1's
            # Newton + normalize run on gpsimd (off the critical path).
            zp = {}

            def emit_norm(b, newton_eng, zmul_eng):
                xr = xs[b].rearrange("c (i r j s) -> c r s i j", r=PS, s=PS,
                                     j=NPH)
                zvs = [xr[:, p, p] for p in range(PS)]
                zsqs = []
                for p in range(PS):
                    zsq = work.tile([128, NP], F32, name="zsq", tag="zsq",
                                    bufs=2)
                    nc.scalar.activation(
                        zsq.rearrange("c (a b) -> c a b", a=NPH), zvs[p],
                        AF.Square)
                    zsqs.append(zsq)
                nrmT = psn.tile([128, 2 * NCHUNK], F32, name="nrmT",
                                tag="nrmT")
                for p in range(PS):
                    for i in range(NCHUNK):
                        nc.tensor.matmul(nrmT[:, 8 * p + i:8 * p + i + 1],
                                         zsqs[p][:, 128 * i:128 * (i + 1)],
                                         ones_k, start=True, stop=True)
                rTn = small.tile([128, 2 * NCHUNK], F32, name="rTn", tag="rTn")
                nc.vector.tensor_copy(rTn, nrmT)
                inv = emit_rsqrt(rTn, F32, "n", 2, newton_eng)
                inv_dram = dsc.tile([PS * NP], F32, name="inv_dram",
                                    tag="inv_dram")
                nc.sync.dma_start(
                    inv_dram[:].rearrange("(c i p) -> p (c i)", c=PS, p=128),
                    inv)
                ibc = work.tile([128, PS, NP], F32, name="ibc", tag="ibc",
                                bufs=2)
                nc.sync.dma_start(
                    ibc.rearrange("c p m -> c (p m)"),
                    inv_dram[:].rearrange("(a m) -> a m", a=1)
                    .partition_broadcast(128))
                for p in range(PS):
                    z = zpool.tile([128, NP], F32R, name=f"z{b}{p}",
                                   tag=f"z{b}{p}", bufs=1)
                    zmul_eng.tensor_tensor(
                        out=z.rearrange("c (a b) -> c a b", a=NPH),
                        in0=zvs[p],
                        in1=ibc[:, p].rearrange("c (a b) -> c a b", a=NPH),
                        op=ALU.mult)
                    zp[(b, p)] = z

            # ---- phase M: six (b, q) stages, software-pipelined.
            s_slabs = {}

            def emit_E(b, q):
                e_slab = epool.tile([128, NCHUNK, NP], F16, name="e_slab",
                                    tag="eslab")
                rT = small.tile([128, NCHUNK], F32, name="rT", tag="rT")
                if q < 2:
                    s_slab = slabp.tile([128, NCHUNK, NP], F16, name="s_slab",
                                        tag="slab16")
                    s_slabs[(b, q)] = s_slab
                    z = zp[(b, q)]
                    for i in range(NCHUNK):
                        G = ps.tile([128, NP], F32, name="G", tag="G")
                        for h in range(2):
                            nc.tensor.matmul(
                                G[:, 512 * h:512 * (h + 1)],
                                z[:, 128 * i:128 * (i + 1)],
                                z[:, 512 * h:512 * (h + 1)],
                                start=True, stop=True)
                        nc.vector.scalar_tensor_tensor(
                            out=s_slab[:, i, :], in0=G, scalar=1.0,
                            in1=mask_sb[:, i, :], op0=ALU.mult, op1=ALU.mult)
                        nc.scalar.activation(e_slab[:, i, :], s_slab[:, i, :],
                                             AF.Exp, scale=scale_E[q],
                                             accum_out=rT[:, i:i + 1])
                else:
                    s0, s1 = s_slabs[(b, 0)], s_slabs[(b, 1)]
                    for i in range(NCHUNK):
                        s2 = work.tile([128, NP], F16, name="s2", tag="s2",
                                       bufs=2)
                        nc.gpsimd.tensor_tensor(out=s2, in0=s0[:, i, :],
                                                in1=s1[:, i, :], op=ALU.add)
                        nc.scalar.activation(e_slab[:, i, :], s2,
                                             AF.Exp, scale=scale_E[q],
                                             accum_out=rT[:, i:i + 1])
                return dict(e_slab=e_slab, rT=rT, b=b, q=q)

            def emit_u_tail(stg):
                """recip + rsqrt + u broadcast for a stage whose E-phase is
                (about to be) complete. Emitted mid-F of the previous stage so
                it neither blocks ready F work in the DVE queue nor cascades
                in the sync DMA queue."""
                uT = small.tile([128, NCHUNK], F32, name="uT", tag="uT")
                nc.vector.reciprocal(uT, stg["rT"])
                rsq = emit_rsqrt(stg["rT"], F16, "u", 1, nc.vector)
                u_dram = dsc.tile([NP], F16, name="u_dram", tag="u_dram")
                nc.sync.dma_start(
                    u_dram[:].rearrange("(i p) -> p i", p=128), rsq)
                squbc = work.tile([128, NP], F16, name="squbc", tag="squbc",
                                  bufs=3)
                nc.sync.dma_start(
                    squbc,
                    u_dram[:].rearrange("(a m) -> a m", a=1)
                    .partition_broadcast(128))
                stg["uT"] = uT
                stg["squbc"] = squbc

            def emit_F(stg, utail_for=None):
                """v = E*sqrt(u[m]) -> top-8 -> (t3^2)*u[n] -> store."""
                b, q = stg["b"], stg["q"]
                t8s = work.tile([128, NCHUNK, 8], F16, name="t8s", tag="t8s",
                                bufs=2)
                for i in range(NCHUNK):
                    v = work.tile([128, NP], F16, name="v", tag="v")
                    nc.vector.tensor_tensor(out=v, in0=stg["e_slab"][:, i, :],
                                            in1=stg["squbc"], op=ALU.mult)
                    nc.vector.max(out=t8s[:, i, :], in_=v)
                if utail_for is not None:
                    emit_u_tail(utail_for)
                w = work.tile([128, NCHUNK, TOPK], F32, name="w", tag="w",
                              bufs=2)
                u3 = stg["uT"].unsqueeze(-1).to_broadcast([128, NCHUNK, TOPK])
                nc.vector.tensor_tensor(out=w, in0=t8s[:, :, :TOPK], in1=u3,
                                        op=ALU.mult)
                oacc = work.tile([128, NCHUNK, TOPK], F32, name="oacc",
                                 tag="oacc", bufs=2)
                nc.vector.tensor_tensor(out=oacc, in0=w, in1=t8s[:, :, :TOPK],
                                        op=ALU.mult)
                dst = out_d[b, q].rearrange("(i p) k -> p i k", p=128)
                nc.sync.dma_start(dst, oacc)

            # ---- emission schedule
            emit_norm(0, nc.vector, nc.vector)   # gates stage 0: low latency
            emit_norm(1, nc.gpsimd, nc.gpsimd)   # off-path
            stages = [(b, q) for b in range(B_LOC) for q in range(3)]
            pending = None
            for (b, q) in stages:
                stg = emit_E(b, q)
                if pending is not None:
                    emit_F(pending, utail_for=stg)
                else:
                    emit_u_tail(stg)
                pending = stg
            emit_F(pending)

    nc.compile()
    return nc


_NC_CACHE = None


def _get_nc():
    global _NC_CACHE
    if _NC_CACHE is None:
        _NC_CACHE = build_nc()
    return _NC_CACHE


def kernel(x: np.ndarray, alpha: np.ndarray) -> np.ndarray:
    global LAST_EXEC_NS
    x = np.ascontiguousarray(np.asarray(x, dtype=np.float32))
    alpha_arr = np.full((128, 1), np.float32(np.asarray(alpha)),
                        dtype=np.float32)
    mask = _build_mask().astype(np.float16)

    nc = _get_nc()
    in_maps = []
    for core in range(N_CORES):
        xsl = x[core * B_LOC:(core + 1) * B_LOC].reshape(B_LOC, C, H * W)
        in_maps.append({"x": np.ascontiguousarray(xsl), "mask": mask,
                        "alpha": alpha_arr})
    res = run_bass_kernel_spmd(nc, in_maps, core_ids=list(range(N_CORES)))
    LAST_EXEC_NS = res.exec_time_ns

    # assemble: out[bg, k, 2i+dr, 2j+dc] from T_q[b, n=i*32+j, k]
    out = np.empty((B_FULL, TOPK, H, W), dtype=np.float32)
    for core in range(N_CORES):
        t = res.results[core]["out"]  # [B_LOC, 3, NP, TOPK]
        for bl in range(B_LOC):
            bg = core * B_LOC + bl
            tq = t[bl].reshape(3, NPH, NPH, TOPK).transpose(0, 3, 1, 2)
            out[bg, :, 0::2, 0::2] = tq[0]
            out[bg, :, 1::2, 1::2] = tq[1]
            out[bg, :, 0::2, 1::2] = tq[2]
            out[bg, :, 1::2, 0::2] = tq[2]
    return out
